# revision 44
# baseline (speedup 1.0000x reference)
"""GPRGNN Trainium2 kernel: MLP + K-hop GCN-normalized propagation + log_softmax.

Self-contained: uses only the container-installed concourse/bass toolchain.
Sharding: nodes destination-sharded across 8 cores (12500/core). Each hop:
  u_{k+1} = deg^{-1} * (scatter_add(gather(u_k, src), dst) + u_k)
with u = D^{1/2}-scaled state so edge messages need no per-edge weight.
Cross-core halo exchange = AllGather of each core's 12544x64 shard per hop.

Edge layout: per core, edges are grouped by (src-superwindow s, dst) and
classed by multiplicity m. Gathers land each dst's m messages on one
partition row across m adjacent columns; the Vector engine column-reduces
them so only ~1 scatter token per (dst, superwindow) remains (~50k/hop vs
200k edges). Issue order is software-pipelined (scatter k-2 after gathers k)
so the in-order GpSimd descriptor-gen queue never stalls on drains.
"""

import sys

sys.path.insert(0, "/opt/trn_rl_repo")

import numpy as np

import concourse.bacc as bacc
import concourse.bass as bass
import concourse.mybir as mybir
import concourse.tile as tile
from concourse.bass_utils import run_bass_kernel_spmd

N = 100000
E = 1600000
FIN = 512
HID = 256
C = 64
K = 10
NCORES = 8
SH = 12500          # real nodes per core
SHP = 12544         # padded shard rows (98 * 128)
SLOTS = SHP // 128  # 98
NT = NCORES * SHP   # full padded table rows
SW = 2 * SHP        # superwindow rows (25088 < int16 max)
NSW = NCORES // 2   # 4 superwindows
HALF = SHP // 2     # dest-half split (6272 = 49*128, slot aligned)
NQ = 4              # SWDGE queues (ucode max)
ACC_NG = 33         # accumulator groups per half: 25 data + 8 trash
CH = 1024           # gather/scatter call token limit (>1024 idxs/call fails)
WCOL = 48           # chunk column budget (SBUF region tile)
MCAP = 6            # runs longer than this are split into <=MCAP pieces
F32 = mybir.dt.float32
BF16 = mybir.dt.bfloat16
I16 = mybir.dt.int16

_cache = {}


def _build(plan, gt, st, temp_vals, nhops=K, do_ag=True, do_gs=True):
    """plan: list of chunks (s, h, m, g, tok_off, scat_off); gt/st total
    gather/scatter tokens."""
    gt16 = gt // 16
    st16 = st // 16
    nc = bacc.Bacc("TRN2", target_bir_lowering=False, debug=False,
                   num_devices=NCORES, num_swdge_queues=NQ)

    xT_h = nc.dram_tensor("xT", [FIN, SHP], BF16, kind="ExternalInput").ap()
    w1_h = nc.dram_tensor("w1", [FIN, HID], BF16, kind="ExternalInput").ap()
    w2_h = nc.dram_tensor("w2", [HID, C], BF16, kind="ExternalInput").ap()
    b1_h = nc.dram_tensor("b1", [HID, 1], F32, kind="ExternalInput").ap()
    b2_h = nc.dram_tensor("b2b", [128, C], F32, kind="ExternalInput").ap()
    dv_h = nc.dram_tensor("dv", [128, SLOTS], F32, kind="ExternalInput").ap()
    d2_h = nc.dram_tensor("d2", [128, SLOTS], F32, kind="ExternalInput").ap()
    dvi_h = nc.dram_tensor("dvi", [128, SLOTS], F32, kind="ExternalInput").ap()
    gi_h = nc.dram_tensor("gidx", [128, gt16], I16, kind="ExternalInput").ap()
    si_h = nc.dram_tensor("sidx", [128, st16], I16, kind="ExternalInput").ap()
    out_h = nc.dram_tensor("out", [SHP, C], F32, kind="ExternalOutput").ap()

    with tile.TileContext(nc, trace_sim=False) as tc:
        with (
            tc.tile_pool(name="persist", bufs=1) as pp,
            tc.tile_pool(name="dram", bufs=1, space="DRAM") as dp,
            tc.tile_pool(name="mlp", bufs=3) as mp,
            tc.tile_pool(name="psum", bufs=2, space="PSUM") as psp,
            tc.tile_pool(name="psum2", bufs=2, space="PSUM") as psp2,
            tc.tile_pool(name="gb", bufs=4) as gp,
            tc.tile_pool(name="rb", bufs=4) as rp,
        ):
            # ---- persistent SBUF ----
            u = pp.tile([128, SLOTS, C], F32)      # local shard state u_k
            Hacc = pp.tile([128, SLOTS, C], F32)   # sum_k temp[k] u_k
            w1sb = pp.tile([128, 4, HID], BF16)
            w2sb = pp.tile([128, 2, C], BF16)
            b1sb = pp.tile([128, 2], F32)
            b2sb = pp.tile([128, C], F32)
            dv = pp.tile([128, SLOTS], F32)
            d2 = pp.tile([128, SLOTS], F32)
            dvi = pp.tile([128, SLOTS], F32)
            gi = pp.tile([128, gt16], I16)
            si = pp.tile([128, st16], I16)
            mx = pp.tile([128, SLOTS], F32)
            sm = pp.tile([128, SLOTS], F32)

            nc.sync.dma_start(w1sb[:], w1_h.rearrange("(k p) h -> p k h", p=128))
            nc.sync.dma_start(w2sb[:], w2_h.rearrange("(k p) f -> p k f", p=128))
            nc.sync.dma_start(b1sb[:], b1_h.rearrange("(k p) o -> p (k o)", p=128))
            nc.sync.dma_start(b2sb[:], b2_h)
            nc.sync.dma_start(dv[:], dv_h)
            nc.sync.dma_start(d2[:], d2_h)
            nc.sync.dma_start(dvi[:], dvi_h)
            nc.sync.dma_start(gi[:], gi_h)
            nc.sync.dma_start(si[:], si_h)

            # ---- internal DRAM ----
            # two half-tables (A=node slots 0..48, B=49..97) per hop: the
            # A-AllGather launches first and B hides under A-window gathers.
            # Shared tiles allow only a single writing instruction.
            NTH = NCORES * HALF  # rows per half-table (8 shards x 6272)
            shr = "Shared" if do_ag is True else "Local"
            tabs = [(dp.tile([NTH, C], F32, addr_space=shr, name=f"tabA{i}"),
                     dp.tile([NTH, C], F32, addr_space=shr, name=f"tabB{i}"))
                    for i in range(max(nhops, 1))]
            aginA = dp.tile([HALF, C], F32)
            aginB = dp.tile([HALF, C], F32)
            # parity-split SBUF accumulators, one pair per dest-half so the
            # two scatter queues share no tile (parallel drains, no WAW chain
            # between queues). Rebased idx: slot 2g -> accE[:, g, :], slot
            # 2g+1 -> accO[:, g, :]; trash (rebased rows HALF..HALF+2047)
            # lands in groups 24..32 which the data never uses.
            accs = [(pp.tile([128, ACC_NG, C], F32, name=f"accE{h}"),
                     pp.tile([128, ACC_NG, C], F32, name=f"accO{h}"))
                    for h in range(2)]

            def wrapped(dram_ap):  # [rows, C] -> [128, rows/128, C] node-major wrap
                return dram_ap.rearrange("(c p) f -> p c f", p=128)

            # ---- MLP: h = relu(x@W1+b1)@W2+b2 ; u0 = dinv*h ----
            moff = 0
            slot = 0
            while moff < SHP:
                mw = min(512, SHP - moff)
                xts = []
                for kk in range(4):
                    xt = mp.tile([128, 512], BF16, tag="xt", bufs=6)
                    nc.sync.dma_start(xt[:, :mw],
                                      xT_h[kk * 128:(kk + 1) * 128,
                                           moff:moff + mw])
                    xts.append(xt)
                h1 = []
                for hb in range(2):
                    ps = psp.tile([128, 512], F32, tag="ps")
                    for kk in range(4):
                        nc.tensor.matmul(ps[:, :mw],
                                         lhsT=w1sb[:, kk, hb * 128:(hb + 1) * 128],
                                         rhs=xts[kk][:, :mw],
                                         start=(kk == 0), stop=(kk == 3))
                    ht = mp.tile([128, 512], BF16, tag="ht")
                    nc.scalar.activation(ht[:, :mw], ps[:, :mw],
                                         mybir.ActivationFunctionType.Relu,
                                         bias=b1sb[:, hb:hb + 1], scale=1.0)
                    h1.append(ht)
                for st_ in range(mw // 128):
                    ps2 = psp2.tile([128, C], F32, tag="ps2")
                    for hb in range(2):
                        nc.tensor.matmul(ps2[:],
                                         lhsT=h1[hb][:, st_ * 128:(st_ + 1) * 128],
                                         rhs=w2sb[:, hb, :],
                                         start=(hb == 0), stop=(hb == 1))
                    t1 = mp.tile([128, C], F32, tag="t1")
                    nc.vector.tensor_add(t1[:], ps2[:], b2sb[:])
                    nc.vector.tensor_scalar(u[:, slot, :], t1[:],
                                            dv[:, slot:slot + 1], None,
                                            mybir.AluOpType.mult)
                    slot += 1
                moff += mw

            # H = temp[0] * u0
            nc.vector.tensor_scalar(Hacc[:], u[:], float(temp_vals[0]), None,
                                    mybir.AluOpType.mult)

            HS = SLOTS // 2  # 49 slots per half

            def halo(k):  # u -> agin halves -> two AllGathers into tabs[k]
                for hb, ag in ((0, aginA), (1, aginB)):
                    sl = slice(0, HS) if hb == 0 else slice(HS, SLOTS)
                    nc.sync.dma_start(wrapped(ag[:, :]), u[:, sl, :])
                    if do_ag is True:
                        nc.gpsimd.collective_compute(
                            "AllGather", mybir.AluOpType.bypass,
                            replica_groups=[list(range(NCORES))],
                            ins=[ag.opt()], outs=[tabs[k][hb].opt()])
                    elif do_ag == "copy":
                        for w in range(NCORES):
                            nc.sync.dma_start(
                                tabs[k][hb][w * HALF:(w + 1) * HALF, :],
                                ag[:, :])

            halo(0)

            # strided u views per (half, parity): (u_slice, ngroups)
            uviews = [[(slice(0, HS, 2), 25), (slice(1, HS, 2), 24)],
                      [(slice(HS, SLOTS, 2), 25), (slice(HS + 1, SLOTS, 2), 24)]]

            WROWS = 4 * HALF  # 25088 rows per window
            for k in range(nhops):
                tabAk, tabBk = tabs[k]
                # accumulator := u_k  (self-loop term), parity-split in SBUF
                for h in range(2):
                    for par, (sl, ng) in enumerate(uviews[h]):
                        nc.vector.tensor_scalar(accs[h][par][:, :ng, :],
                                                u[:, sl, :], 1.0, None,
                                                mybir.AluOpType.mult)
                if do_gs:
                    # queue load balancing: scatters pinned to queue 2+h,
                    # gather calls go to the least-loaded queue
                    load = [0, 0, st // 2, st // 2]
                    pend = []  # deferred scatters: (src_ap, h, scat_off, ntok)

                    def flush_one():
                        rsrc, h, soff, ntok = pend.pop(0)
                        nc.gpsimd.dma_scatter_add(
                            accs[h][0][:, :, :], rsrc,
                            si[:, soff // 16:(soff + ntok) // 16],
                            ntok, ntok, C,
                            sbuf_tokens_per_rank=128, parity_reg=0,
                            out_ap_other=accs[h][1][:, :, :],
                            queue_num=2 + h)

                    for (s, h, m, g2, toff, soff) in plan:
                        srcw = (tabAk[s * WROWS:(s + 1) * WROWS, :] if s < 2
                                else tabBk[(s - 2) * WROWS:(s - 1) * WROWS, :])
                        cols = g2 * m
                        reg = gp.tile([128, WCOL, C], F32, tag="reg")
                        for q in range(0, cols, CH // 128):
                            kk = min(CH // 128, cols - q)
                            n = kk * 128
                            t0 = toff + q * 128
                            qn = min(range(NQ), key=lambda i: load[i])
                            load[qn] += n
                            nc.gpsimd.dma_gather(
                                reg[:, q:q + kk, :], srcw,
                                gi[:, t0 // 16:(t0 + n) // 16],
                                n, n, C, queue_num=qn)
                        if do_gs == "gonly":
                            continue
                        if m == 1:
                            rsrc = reg[:, :cols, :]
                        else:
                            red = rp.tile([128, CH // 128, C], F32, tag="red")
                            v4 = reg[:, 0:cols, :].rearrange(
                                "p (g m) f -> p g m f", m=m)
                            nc.vector.tensor_tensor(
                                red[:, :g2, :], v4[:, :, 0, :], v4[:, :, 1, :],
                                mybir.AluOpType.add)
                            for i in range(2, m):
                                nc.vector.tensor_tensor(
                                    red[:, :g2, :], red[:, :g2, :],
                                    v4[:, :, i, :], mybir.AluOpType.add)
                            rsrc = red[:, :g2, :]
                        if do_gs == "nosc":
                            continue
                        pend.append((rsrc, h, soff, g2 * 128))
                        if len(pend) > 2:
                            flush_one()
                    while pend:
                        flush_one()
                # u_{k+1} = deg^-1 * acc ; per half: update u then launch its
                # AllGather immediately (A first so next hop's A-window
                # gathers unblock soonest); Hacc update last (off chain)
                for h in range(2):
                    for par, (sl, ng) in enumerate(uviews[h]):
                        nc.vector.tensor_tensor(
                            u[:, sl, :], accs[h][par][:, :ng, :],
                            d2[:, sl, None].to_broadcast([128, ng, C]),
                            mybir.AluOpType.mult)
                    if k < nhops - 1:
                        ag = aginA if h == 0 else aginB
                        sl2 = slice(0, HS) if h == 0 else slice(HS, SLOTS)
                        nc.sync.dma_start(wrapped(ag[:, :]), u[:, sl2, :])
                        if do_ag is True:
                            nc.gpsimd.collective_compute(
                                "AllGather", mybir.AluOpType.bypass,
                                replica_groups=[list(range(NCORES))],
                                ins=[ag.opt()], outs=[tabs[k + 1][h].opt()])
                        elif do_ag == "copy":
                            for w in range(NCORES):
                                nc.sync.dma_start(
                                    tabs[k + 1][h][w * HALF:(w + 1) * HALF, :],
                                    ag[:, :])
                nc.vector.scalar_tensor_tensor(
                    Hacc[:], u[:], float(temp_vals[k + 1]), Hacc[:],
                    mybir.AluOpType.mult, mybir.AluOpType.add)

            # ---- hidden = H * dinv^-1 ; log_softmax (in place over u) ----
            nc.vector.tensor_tensor(
                u[:], Hacc[:],
                dvi[:, :, None].to_broadcast([128, SLOTS, C]),
                mybir.AluOpType.mult)
            nc.vector.tensor_reduce(mx[:], u[:], mybir.AxisListType.X,
                                    mybir.AluOpType.max)
            nc.vector.tensor_tensor(
                u[:], u[:],
                mx[:, :, None].to_broadcast([128, SLOTS, C]),
                mybir.AluOpType.subtract)
            s0 = 0
            while s0 < SLOTS:
                sn = min(WCOL, SLOTS - s0)
                ex = gp.tile([128, WCOL, C], F32, tag="reg")
                nc.scalar.activation(ex[:, :sn, :], u[:, s0:s0 + sn, :],
                                     mybir.ActivationFunctionType.Exp)
                nc.vector.tensor_reduce(sm[:, s0:s0 + sn], ex[:, :sn, :],
                                        mybir.AxisListType.X,
                                        mybir.AluOpType.add)
                s0 += sn
            nc.scalar.activation(sm[:], sm[:],
                                 mybir.ActivationFunctionType.Ln)
            nc.vector.tensor_tensor(
                u[:], u[:],
                sm[:, :, None].to_broadcast([128, SLOTS, C]),
                mybir.AluOpType.subtract)
            nc.sync.dma_start(wrapped(out_h), u[:])

    nc.compile()
    return nc


def _preprocess(edge_index):
    """Group each core's edges by (src-superwindow s, dst); class = run
    length m (capped). Class (s, m) lays its dsts 128-per-block with the m
    messages of one dst on one partition row across m adjacent columns, so a
    single strided vector add per column folds duplicates before scatter.
    Chunks of g blocks (g*m <= WCOL) are the pipeline/scatter unit; all
    chunk shapes are shared across cores (max-padded) so the SPMD program
    is identical and only the index data differs.

    Returns dinv, deg, per-core wrapped gidx/sidx, plan, gt, st.
    """
    row = np.asarray(edge_index[0], dtype=np.int64)
    col = np.asarray(edge_index[1], dtype=np.int64)
    deg = (np.bincount(col, minlength=N) + 1.0).astype(np.float32)
    dinv = deg ** -0.5

    core = col // SH
    # half-split tables: node n -> half hb=(loc>=HALF), row (shard*HALF +
    # loc%HALF) in table hb. Window s: {0,1}=A-half shard quads, {2,3}=B.
    shard = row // SH
    loc = row % SH
    hb = loc >= HALF
    s = shard // 4 + 2 * hb
    wloc = ((shard % 4) * HALF + loc % HALF).astype(np.int16)

    key = (core * NSW + s) * N + col
    order = np.argsort(key, kind="stable")
    ks = key[order]
    newrun = np.r_[True, ks[1:] != ks[:-1]]
    run_start = np.flatnonzero(newrun)
    run_len = np.diff(np.r_[run_start, E])
    rc = core[order][run_start]
    rs = s[order][run_start]
    rd = (col[order][run_start] % SH).astype(np.int16)

    # Split long runs into pieces of strictly decreasing size (MCAP, MCAP-1,
    # ...) so no two pieces of one (core, s, dst) share a class -> never in
    # the same scatter call (the CCE add loses intra-call collisions).
    parts = []
    rem_start, rem_len = run_start, run_len
    rem_c, rem_s, rem_d = rc, rs, rd
    size = MCAP
    while len(rem_start):
        assert size >= 1, f"run too long for distinct split: {rem_len.max()}"
        take = np.minimum(rem_len, size)
        parts.append((rem_start, take, rem_c, rem_s, rem_d))
        more = rem_len > size
        rem_start = rem_start[more] + size
        rem_len = rem_len[more] - size
        rem_c, rem_s, rem_d = rem_c[more], rem_s[more], rem_d[more]
        size -= 1
    p_start = np.concatenate([p[0] for p in parts])
    p_len = np.concatenate([p[1] for p in parts])
    p_c = np.concatenate([p[2] for p in parts])
    p_s = np.concatenate([p[3] for p in parts])
    p_d = np.concatenate([p[4] for p in parts])

    p_h = (p_d >= HALF).astype(np.int64)

    # class sizes: P[s][h][m] = 128-padded max over cores of piece count
    cnt = np.zeros((NCORES, NSW, 2, MCAP + 1), np.int64)
    np.add.at(cnt, (p_c, p_s, p_h, p_len), 1)
    cmax = cnt.max(axis=0)                     # [NSW, 2, MCAP+1]
    P = ((cmax + 127) // 128) * 128

    # chunk plan (shared across cores); A-half windows (s 0,1) first so the
    # B-half AllGather hides under their gathers
    plan = []
    # block b of class (s, h, m) -> (tok_off of its chunk, scat_off, b0)
    blk_info = {}
    gt = 0
    st = 0
    for sw in range(NSW):
        for hh in range(2):
            for m in range(1, MCAP + 1):
                B = int(P[sw, hh, m]) // 128
                if B == 0:
                    continue
                g = max(1, min(CH // 128, WCOL // m))
                b0 = 0
                while b0 < B:
                    g2 = min(g, B - b0)
                    plan.append((sw, hh, m, g2, gt, st))
                    for bb in range(g2):
                        blk_info[(sw, hh, m, b0 + bb)] = (gt, st, b0)
                    gt += g2 * m * 128
                    st += g2 * 128
                    b0 += g2
    assert gt % 16 == 0 and st % 16 == 0

    gidx = np.zeros((NCORES, gt), np.int16)
    sidx = np.empty((NCORES, st), np.int16)
    # default: rebased trash rows (HALF + x lands in groups 24..32 of either
    # half's accumulators; distinct within any 1024-token call)
    sidx[:] = (HALF + (np.arange(st) % 2048)).astype(np.int16)

    wl_o = wloc[order]
    # per (core, s, h, m): fill this core's pieces into class slots
    piece_key = ((p_c * NSW + p_s) * 2 + p_h) * (MCAP + 1) + p_len
    porder = np.argsort(piece_key, kind="stable")
    pk_s = piece_key[porder]
    pb = np.r_[True, pk_s[1:] != pk_s[:-1]]
    grp_start = np.flatnonzero(pb)
    grp_len = np.diff(np.r_[grp_start, len(porder)])
    for gs0, gl in zip(grp_start, grp_len):
        idxs = porder[gs0:gs0 + gl]
        cc = int(p_c[idxs[0]]); sw = int(p_s[idxs[0]])
        hh = int(p_h[idxs[0]]); m = int(p_len[idxs[0]])
        j = np.arange(gl)
        b = j // 128
        starts = p_start[idxs]
        info = np.array([blk_info[(sw, hh, m, int(bb))] for bb in b])
        chunk_off, scat_off, b0 = info[:, 0], info[:, 1], info[:, 2]
        tpos = (chunk_off[:, None]
                + (((b - b0) * m)[:, None] + np.arange(m)[None, :]) * 128
                + (j % 128)[:, None])
        ev = wl_o[starts[:, None] + np.arange(m)[None, :]]
        gidx[cc, tpos.ravel()] = ev.ravel()
        spos = scat_off + (b - b0) * 128 + (j % 128)
        sidx[cc, spos] = p_d[idxs] - hh * HALF

    def wrap(a):  # [L] -> [128, L/16] with token j at [j%16, j//16], 8x replicated
        return np.tile(a.reshape(-1, 16).T, (NCORES, 1)).copy()

    gw = [wrap(gidx[c]) for c in range(NCORES)]
    sw_ = [wrap(sidx[c]) for c in range(NCORES)]
    return dinv, deg, gw, sw_, plan, gt, st


def _make_in_maps(inputs, dinv, deg, gw, sw_):
    import ml_dtypes
    bf16 = ml_dtypes.bfloat16
    x = np.asarray(inputs["x"], dtype=np.float32)
    W1 = np.asarray(inputs["W1"], dtype=np.float32).astype(bf16)
    b1 = np.asarray(inputs["b1"], dtype=np.float32)
    W2 = np.asarray(inputs["W2"], dtype=np.float32).astype(bf16)
    b2 = np.asarray(inputs["b2"], dtype=np.float32)
    b2b = np.broadcast_to(b2[None, :], (128, C)).copy()
    in_maps = []
    for core in range(NCORES):
        lo = core * SH
        xs = np.zeros((SHP, FIN), np.float32)
        xs[:SH] = x[lo:lo + SH]
        dloc = np.zeros(SHP, np.float32)
        dloc[:SH] = dinv[lo:lo + SH]
        d2loc = np.zeros(SHP, np.float32)
        d2loc[:SH] = 1.0 / deg[lo:lo + SH]
        dviloc = np.zeros(SHP, np.float32)
        dviloc[:SH] = np.sqrt(deg[lo:lo + SH])

        def wrapv(v):  # [SHP] -> [128, SLOTS] with node n at [n%128, n//128]
            return v.reshape(SLOTS, 128).T.copy()

        in_maps.append({
            "xT": np.ascontiguousarray(xs.T).astype(bf16),
            "w1": W1, "w2": W2,
            "b1": b1[:, None].copy(), "b2b": b2b,
            "dv": wrapv(dloc), "d2": wrapv(d2loc), "dvi": wrapv(dviloc),
            "gidx": gw[core], "sidx": sw_[core],
        })
    return in_maps


def kernel(**inputs):
    edge_index = np.asarray(inputs["edge_index"])
    temp = np.asarray(inputs["temp"], dtype=np.float32)

    dinv, deg, gw, sw_, plan, gt, st = _preprocess(edge_index)

    key = (gt, st, tuple(plan), tuple(np.round(temp, 10)))
    if key not in _cache:
        _cache[key] = _build(plan, gt, st, [float(t) for t in temp])
    nc = _cache[key]

    in_maps = _make_in_maps(inputs, dinv, deg, gw, sw_)
    res = run_bass_kernel_spmd(nc, in_maps, list(range(NCORES)))
    outs = [res.results[c]["out"] for c in range(NCORES)]
    return np.concatenate([o[:SH] for o in outs], axis=0)
